# revision 31
# baseline (speedup 1.0000x reference)
"""Causal self-attention (B=2, T=2048, C=2048, H=16, D=128) on 8 TRN2 NeuronCores.

Sharding: 8 cores = 2 batches x 4 head-groups (4 heads each).
Core m: batch b = m // 4, heads [4g, 4g+4) with g = m % 4.
  - c_attn columns split by head (tensor parallel), c_proj rows split by head.
  - Each core returns a partial projection output; host sums the 4 partials
    per batch and adds b_proj (the unshard step for row-parallel c_proj).

Host prep: x is transposed on the host (layout prep) and cast to bf16, so the
device skips the on-chip transpose entirely. W_attn/W_proj also bf16.

Per-core pipeline:
  A:  QT/KT/VT = (x @ W)^T via bf16 matmuls (stationary W chunk, moving xT),
      evacuated by ACT straight into SBUF-resident f32r tiles (no DRAM scratch).
      V goes to bf16 staging tiles for the transpose step.
  T:  per head: transpose VT -> V chunks [k, d] via wide-identity bf16 matmuls.
  B:  qb-outer, head-inner flash-ish attention: ST = K Q^T chunk [128k, 512q]
      -> exp on ACT -> (mask-mul on DVE for the 4 diagonal chunks, processed
      FIRST so their longer chain hides behind clean chunks) -> yT += V^T P,
      sums += ones^T P on PE. Tail: 1/sum via DVE reciprocal (no Ln -> no ACT
      table thrash), broadcast via GpSimd partition_broadcast, normalize on DVE
      into bf16 ytc tiles. Emission software-pipelined with a lag queue.
  C:  out = concat_heads(y) @ Wp_rows (bf16), emitted per-qb as soon as that
      qb's 4 heads are normalized -> interleaves with the next qb's B work so
      the PE never drains between B and C.
"""
import sys

sys.path.insert(0, "/opt/trn_rl_repo")
sys.path.insert(0, "/root/.axon_site")

import numpy as np

N_EMBD = 2048
N_HEAD = 16
HEAD_DIM = 128
B, T = 2, 2048
N_CORES = 8
H_PER_CORE = 4          # heads per core
HD = H_PER_CORE * HEAD_DIM  # 512: per-core q/k/v width
NC_C = N_EMBD // 128    # 16 contraction chunks
NT = T // 128           # 16 token 128-blocks
NQB = T // 512          # 4 q blocks of 512
SCALE = 1.0 / np.sqrt(HEAD_DIM)
LAG = 3                 # deferred-emission lag (chunks in flight)

_CACHE = {}


def _build():
    import concourse.bacc as bacc
    import concourse.mybir as mybir
    import concourse.tile as tile

    f32 = mybir.dt.float32
    f32r = mybir.dt.float32r
    bf16 = mybir.dt.bfloat16
    Exp = mybir.ActivationFunctionType.Exp
    Ident = mybir.ActivationFunctionType.Identity

    nc = bacc.Bacc("TRN2", target_bir_lowering=False, debug=False, num_devices=N_CORES)

    xt_dram = nc.dram_tensor("xt", [N_EMBD, T], bf16, kind="ExternalInput").ap()
    # host-packed: wa[od][p, c*128+j] = W_{q,k}[c*128+p, hh*128+j], od = kind*4+hh
    # (contiguous 4KB rows -> big DMA descriptors; column-slices of W would be
    # 256B descriptors that crawl under fair-share DMA scheduling)
    wa_dram = nc.dram_tensor("wa", [2 * H_PER_CORE, 128, NC_C * 128], bf16,
                             kind="ExternalInput").ap()
    # host-packed: wvp[p, c*512+j] = W_v[c*128+p, j]
    wvp_dram = nc.dram_tensor("wvp", [128, NC_C * HD], bf16,
                              kind="ExternalInput").ap()
    bq_dram = nc.dram_tensor("bq", [HD, 1], f32, kind="ExternalInput").ap()
    bk_dram = nc.dram_tensor("bk", [HD, 1], f32, kind="ExternalInput").ap()
    bv_dram = nc.dram_tensor("bv", [1, HD], f32, kind="ExternalInput").ap()
    wp_dram = nc.dram_tensor("wp", [HD, N_EMBD], bf16, kind="ExternalInput").ap()
    ones_dram = nc.dram_tensor("ones", [128, 1], bf16, kind="ExternalInput").ap()
    mmask_dram = nc.dram_tensor("mmask", [128, 4, 512], bf16,
                                kind="ExternalInput").ap()
    out_dram = nc.dram_tensor("out", [T, N_EMBD], f32, kind="ExternalOutput").ap()

    with tile.TileContext(nc) as tc:
        with tc.tile_pool(name="singles", bufs=1) as singles, \
             tc.tile_pool(name="qk", bufs=1) as qk_pool, \
             tc.tile_pool(name="vh", bufs=1) as vh_pool:

            ones_col = singles.tile([128, 1], bf16)
            nc.sync.dma_start(ones_col[:], ones_dram[:])
            bias_t = singles.tile([128, 2 * H_PER_CORE], f32)
            nc.sync.dma_start(bias_t[:, 0:4], bq_dram.rearrange("(a p) o -> p (a o)", p=128))
            nc.sync.dma_start(bias_t[:, 4:8], bk_dram.rearrange("(a p) o -> p (a o)", p=128))
            bv_t = singles.tile([1, HD], f32)
            nc.sync.dma_start(bv_t[:], bv_dram[:])

            # resident K^T / Q^T per head [128 d, T], f32r
            qt = [qk_pool.tile([128, T], f32r, tag=f"qt{h}", name=f"qt{h}")
                  for h in range(H_PER_CORE)]
            kt = [qk_pool.tile([128, T], f32r, tag=f"kt{h}", name=f"kt{h}")
                  for h in range(H_PER_CORE)]
            # resident V chunks [128 k, (kc, d)] per head, bf16 (PV runs bf16)
            v_h = [vh_pool.tile([128, NT, 128], bf16, tag=f"vh{h}", name=f"vh{h}")
                   for h in range(H_PER_CORE)]

            # ------- Phase A: K/Q projections, then V in [t, d] blocks -------
            with tc.tile_pool(name="xt", bufs=1) as xt_pool, \
                 tc.tile_pool(name="wqkv", bufs=2) as wqkv_pool, \
                 tc.tile_pool(name="wv", bufs=1) as wv_pool, \
                 tc.tile_pool(name="psA", bufs=2, space="PSUM") as psA:
                # k first (B needs full K per head), then q
                od_order = [(1, hh) for hh in range(H_PER_CORE)] + \
                           [(0, hh) for hh in range(H_PER_CORE)]

                w_tiles = {}

                def issue_w(oi, eng):
                    # one contiguous-row DMA per od (host pre-packed layout)
                    w = wqkv_pool.tile([128, NC_C, 128], bf16, tag="wqkv",
                                       name=f"w{oi}")
                    kind, hh = od_order[oi]
                    od = kind * H_PER_CORE + hh
                    eng.dma_start(
                        w[:], wa_dram[od].rearrange("p (c j) -> p c j", j=128))
                    w_tiles[oi] = w

                # w0/w1 before the bulk x transfer; later ods JIT via the
                # scalar queue (gated behind evac work -> don't steal early
                # DMA bandwidth from the x stream)
                issue_w(0, nc.sync)
                issue_w(1, nc.sync)
                # x tiles on the gpsimd DMA queue, decoupled from the sync queue
                xt = []
                for c in range(NC_C):
                    t = xt_pool.tile([128, T], bf16, tag=f"xt{c}", name=f"xt{c}")
                    nc.gpsimd.dma_start(t[:], xt_dram[c * 128:(c + 1) * 128, :])
                    xt.append(t)
                # broadcast the V bias to all partitions: needed by the V-phase
                # evac, and doubles as the GPSIMD library preload (first custom
                # ISA op pays a multi-us Q7 program fetch). Placed AFTER the x
                # DMA configs so the load runs during phase A, not ahead of
                # the x stream.
                bvb = singles.tile([128, HD], f32)
                nc.gpsimd.partition_broadcast(bvb[:], bv_t[:])

                def evac_od(oi, psums):
                    kind, hh = od_order[oi]
                    od = kind * H_PER_CORE + hh
                    dst = (qt[hh], kt[hh])[kind]
                    for tqb in range(NQB):
                        nc.scalar.activation(
                            dst[:, tqb * 512:(tqb + 1) * 512],
                            psums[tqb][:], Ident,
                            bias=bias_t[:, od:od + 1], scale=1.0)

                wv_all = wv_pool.tile([128, NC_C, HD], bf16, tag="wv",
                                      name="wv_all")
                # first two ods c-interleaved (their 8 psums fill all banks):
                # ~1.7us of PE work per x-tile arrival keeps the PE fed while
                # the x stream is the limiter
                ps_pair = []
                for oi in (0, 1):
                    ps_pair.append([psA.tile([128, 512], f32, tag=f"qk{t}",
                                             name=f"pp{oi}_{t}")
                                    for t in range(NQB)])
                for c in range(NC_C):
                    for oi in (0, 1):
                        w = w_tiles[oi]
                        for tqb in range(NQB):
                            nc.tensor.matmul(
                                ps_pair[oi][tqb][:], w[:, c, :],
                                xt[c][:, tqb * 512:(tqb + 1) * 512],
                                start=(c == 0), stop=(c == NC_C - 1),
                            )
                for oi in (0, 1):
                    evac_od(oi, ps_pair[oi])
                    del w_tiles[oi]

                issue_w(2, nc.sync)
                issue_w(3, nc.scalar)
                for oi in range(2, len(od_order)):
                    kind, hh = od_order[oi]
                    if oi + 2 < len(od_order):
                        issue_w(oi + 2, nc.scalar)
                    if oi == len(od_order) - 2:
                        # Wv row-blocks: emitted here so the scalar SEQ gates
                        # the 2MB transfer behind mid-A evac work
                        nc.scalar.dma_start(
                            wv_all[:],
                            wvp_dram.rearrange("p (c j) -> p c j", j=HD))
                    od = kind * H_PER_CORE + hh
                    psums = []
                    for tqb in range(NQB):
                        p = psA.tile([128, 512], f32, tag=f"qk{tqb}",
                                     name=f"qk{od}_{tqb}")
                        psums.append(p)
                    for c in range(NC_C):
                        w = w_tiles[oi]
                        for tqb in range(NQB):
                            nc.tensor.matmul(
                                psums[tqb][:], w[:, c, :],
                                xt[c][:, tqb * 512:(tqb + 1) * 512],
                                start=(c == 0), stop=(c == NC_C - 1),
                            )
                    del w_tiles[oi]
                    evac_od(oi, psums)

                # V phase: out[t, d] blocks — stationary xt slice, moving Wv
                # row-block. Evacuates straight into the [k, d] layout the PV
                # matmul wants (no transpose), bias added during the evac.
                for tb in range(NT):
                    pv = psA.tile([128, 512], f32, tag=f"qk{tb % 4}",
                                  name=f"pv{tb}")
                    for c in range(NC_C):
                        nc.tensor.matmul(
                            pv[:], xt[c][:, tb * 128:(tb + 1) * 128],
                            wv_all[:, c, :],
                            start=(c == 0), stop=(c == NC_C - 1),
                        )
                    for h in range(H_PER_CORE):
                        with nc.allow_low_precision(reason="v evac f32r"):
                            nc.vector.tensor_add(
                                v_h[h][:, tb, :],
                                pv[:, h * 128:(h + 1) * 128],
                                bvb[:, h * 128:(h + 1) * 128])

            # ---------------- Phases B & C (interleaved) ----------------
            with tc.tile_pool(name="wp", bufs=1) as wp_pool, \
                 tc.tile_pool(name="ytcp", bufs=1) as ytc_pool, \
                 tc.tile_pool(name="bconst", bufs=1) as bconst, \
                 tc.tile_pool(name="pt", bufs=6) as pt_pool, \
                 tc.tile_pool(name="ptm", bufs=4) as ptm_pool, \
                 tc.tile_pool(name="small", bufs=3) as small_pool, \
                 tc.tile_pool(name="oev", bufs=4) as oev_pool, \
                 tc.tile_pool(name="psM", bufs=5, space="PSUM") as psM, \
                 tc.tile_pool(name="psY", bufs=2, space="PSUM") as psY, \
                 tc.tile_pool(name="psS", bufs=1, space="PSUM") as psS:

                mmask = bconst.tile([128, 4, 512], bf16)
                nc.sync.dma_start(mmask[:], mmask_dram[:])
                wp_t = []
                for h in range(H_PER_CORE):
                    w = wp_pool.tile([128, N_EMBD], bf16, tag=f"wp{h}", name=f"wp{h}")
                    nc.sync.dma_start(w[:], wp_dram[h * 128:(h + 1) * 128, :])
                    wp_t.append(w)
                ytc = [ytc_pool.tile([128, 512], bf16, tag=f"ytc{i}", name=f"ytc{i}")
                       for i in range(H_PER_CORE * NQB)]

                deferred = []

                def flush(keep):
                    while len(deferred) > keep:
                        deferred.pop(0)()

                # qb1 first (its short-tail blocks hide behind qb2/qb3 work),
                # qb0 last so the final C tail is the small 4-block one and
                # qb0's B hides C(qb3)'s flushes
                for qi, qb in enumerate((1, 2, 3, 0)):
                    for h in range(H_PER_CORE):
                        i = h * NQB + qb
                        nkc = 4 * (qb + 1)
                        # diagonal (masked) chunks first: their exp->mask chain
                        # is longer; clean chunks behind them keep the PE fed
                        kc_order = list(range(4 * qb, 4 * qb + 4)) + list(range(4 * qb))
                        yt_ps = psY.tile([128, 512], f32, tag="yt", name=f"yt{h}_{qb}")
                        sum_ps = psS.tile([1, 512], f32, tag="sum", name=f"sum{h}_{qb}")
                        for idx, kc in enumerate(kc_order):
                            st = psM.tile([128, 512], f32, tag="mm",
                                          name=f"st{h}_{qb}_{kc}")
                            nc.tensor.matmul(
                                st[:], kt[h][:, kc * 128:(kc + 1) * 128],
                                qt[h][:, qb * 512:(qb + 1) * 512],
                                start=True, stop=True)
                            pt = pt_pool.tile([128, 512], bf16, tag="pt",
                                              name=f"pt{h}_{qb}_{kc}")
                            with nc.allow_low_precision(reason="exp bf16"):
                                nc.scalar.activation(pt[:], st[:], Exp, scale=SCALE)
                            if kc >= 4 * qb:  # diagonal: multiplicative causal mask
                                o = kc - 4 * qb
                                ptm = ptm_pool.tile([128, 512], bf16, tag="ptm",
                                                    name=f"ptm{h}_{qb}_{kc}")
                                with nc.allow_low_precision(reason="mask mul bf16"):
                                    nc.vector.tensor_mul(ptm[:], pt[:], mmask[:, o, :])
                                src = ptm
                            else:
                                src = pt

                            def consume(src=src, yt_ps=yt_ps, sum_ps=sum_ps, kc=kc,
                                        idx=idx, nkc=nkc, h_=h,
                                        last=(idx == nkc - 1)):
                                nc.tensor.matmul(
                                    yt_ps[:], v_h[h_][:, kc, :], src[:],
                                    start=(idx == 0), stop=last)
                                nc.tensor.matmul(
                                    sum_ps[:], ones_col[:], src[:],
                                    start=(idx == 0), stop=last)

                            deferred.append(consume)
                            flush(keep=LAG)

                        def tail(i=i, yt_ps=yt_ps, sum_ps=sum_ps, h_=h, qb_=qb):
                            rinv = small_pool.tile([1, 512], f32, tag="rinv",
                                                   name=f"ri{h_}_{qb_}")
                            nc.vector.reciprocal_approx_fast(rinv[:], sum_ps[:])
                            rbc = small_pool.tile([128, 512], f32, tag="rbc",
                                                  name=f"rb{h_}_{qb_}")
                            nc.gpsimd.partition_broadcast(rbc[:], rinv[:])
                            with nc.allow_low_precision(reason="softmax norm bf16"):
                                nc.vector.tensor_mul(ytc[i][:], yt_ps[:], rbc[:])

                        deferred.append(tail)
                        flush(keep=LAG)

                    # C for this qb: emitted now, drains interleaved with the
                    # next qb's B chunks via the lag queue
                    for tb in range(qb * 4, qb * 4 + 4):
                        ts = (tb % 4) * 128
                        oev = oev_pool.tile([128, N_EMBD], f32, tag="oev",
                                            name=f"oev{tb}")
                        for ob in range(4):
                            def cblock(tb=tb, ts=ts, ob=ob, qb_=qb, oev=oev):
                                po = psM.tile([128, 512], f32, tag="mm",
                                              name=f"po{tb}_{ob}")
                                for h in range(H_PER_CORE):
                                    nc.tensor.matmul(
                                        po[:],
                                        ytc[h * NQB + qb_][:, ts:ts + 128],
                                        wp_t[h][:, ob * 512:(ob + 1) * 512],
                                        start=(h == 0), stop=(h == H_PER_CORE - 1))
                                if ob % 2 == 0:
                                    nc.scalar.copy(
                                        oev[:, ob * 512:(ob + 1) * 512], po[:])
                                else:
                                    nc.vector.tensor_copy(
                                        oev[:, ob * 512:(ob + 1) * 512], po[:])
                                if ob == 3:
                                    nc.sync.dma_start(
                                        out_dram[tb * 128:(tb + 1) * 128, :], oev[:])

                            deferred.append(cblock)
                            if qi < NQB - 1:
                                flush(keep=LAG)
                flush(keep=0)

    nc.compile()
    return nc


def _consts():
    # mask[k_local, o, q_local] = 1 iff q_local >= o*128 + k_local
    # (chunk kc = 4*qb + o; keep iff global q >= global k, qb-independent)
    kk = np.arange(128)[:, None, None]
    oo = np.arange(4)[None, :, None]
    qq = np.arange(512)[None, None, :]
    mmask = (qq >= oo * 128 + kk).astype(np.float32)
    import ml_dtypes
    bf16 = ml_dtypes.bfloat16
    return {
        "ones": np.ones((128, 1), np.float32).astype(bf16),
        "mmask": mmask.astype(bf16),
    }


def _run(inputs, trace=False):
    import ml_dtypes
    from concourse.bass_utils import run_bass_kernel_spmd

    bf16 = ml_dtypes.bfloat16
    if "nc" not in _CACHE:
        _CACHE["nc"] = _build()
    nc = _CACHE["nc"]

    x = np.asarray(inputs["x"], dtype=np.float32)
    W_attn = np.asarray(inputs["W_attn"], dtype=np.float32)
    b_attn = np.asarray(inputs["b_attn"], dtype=np.float32)
    W_proj = np.asarray(inputs["W_proj"], dtype=np.float32)
    b_proj = np.asarray(inputs["b_proj"], dtype=np.float32)

    consts = _consts()
    xt_b = [np.ascontiguousarray(x[b].T).astype(bf16) for b in range(B)]
    in_maps = []
    for m in range(N_CORES):
        b, g = m // 4, m % 4
        cs = g * HD
        # wa[od] = [128 p, 16 c * 128 j] with od = kind*4+hh, kind 0=q / 1=k
        wa = np.empty((2 * H_PER_CORE, 128, NC_C * 128), dtype=bf16)
        for kind, base in ((0, cs), (1, N_EMBD + cs)):
            Ws = W_attn[:, base:base + HD]  # [2048, 512] f32
            for hh in range(H_PER_CORE):
                blk = Ws[:, hh * 128:(hh + 1) * 128].reshape(NC_C, 128, 128)
                wa[kind * H_PER_CORE + hh] = blk.transpose(1, 0, 2).reshape(
                    128, NC_C * 128).astype(bf16)
        Wv = W_attn[:, 2 * N_EMBD + cs:2 * N_EMBD + cs + HD]
        wvp = Wv.reshape(NC_C, 128, HD).transpose(1, 0, 2).reshape(
            128, NC_C * HD).astype(bf16)
        im = {
            "xt": xt_b[b],
            "wa": wa,
            "wvp": np.ascontiguousarray(wvp),
            "bq": np.ascontiguousarray(b_attn[cs:cs + HD].reshape(HD, 1)),
            "bk": np.ascontiguousarray(
                b_attn[N_EMBD + cs:N_EMBD + cs + HD].reshape(HD, 1)),
            "bv": np.ascontiguousarray(
                b_attn[2 * N_EMBD + cs:2 * N_EMBD + cs + HD].reshape(1, HD)),
            "wp": np.ascontiguousarray(W_proj[cs:cs + HD, :]).astype(bf16),
        }
        im.update(consts)
        in_maps.append(im)

    res = run_bass_kernel_spmd(nc, in_maps, list(range(N_CORES)), trace=trace)
    out = np.zeros((B, T, N_EMBD), dtype=np.float32)
    for m in range(N_CORES):
        out[m // 4] += res.results[m]["out"]
    out += b_proj
    return out, res


def kernel(**inputs) -> np.ndarray:
    out, _ = _run(inputs, trace=False)
    return out


# revision 33
# speedup vs baseline: 1.0114x; 1.0114x over previous
"""Causal self-attention (B=2, T=2048, C=2048, H=16, D=128) on 8 TRN2 NeuronCores.

Sharding: 8 cores = 2 batches x 4 head-groups (4 heads each).
Core m: batch b = m // 4, heads [4g, 4g+4) with g = m % 4.
  - c_attn columns split by head (tensor parallel), c_proj rows split by head.
  - Each core returns a partial projection output; host sums the 4 partials
    per batch and adds b_proj (the unshard step for row-parallel c_proj).

Host prep: x is transposed on the host (layout prep) and cast to bf16, so the
device skips the on-chip transpose entirely. W_attn/W_proj also bf16.

Per-core pipeline:
  A:  QT/KT/VT = (x @ W)^T via bf16 matmuls (stationary W chunk, moving xT),
      evacuated by ACT straight into SBUF-resident f32r tiles (no DRAM scratch).
      V goes to bf16 staging tiles for the transpose step.
  T:  per head: transpose VT -> V chunks [k, d] via wide-identity bf16 matmuls.
  B:  qb-outer, head-inner flash-ish attention: ST = K Q^T chunk [128k, 512q]
      -> exp on ACT -> (mask-mul on DVE for the 4 diagonal chunks, processed
      FIRST so their longer chain hides behind clean chunks) -> yT += V^T P,
      sums += ones^T P on PE. Tail: 1/sum via DVE reciprocal (no Ln -> no ACT
      table thrash), broadcast via GpSimd partition_broadcast, normalize on DVE
      into bf16 ytc tiles. Emission software-pipelined with a lag queue.
  C:  out = concat_heads(y) @ Wp_rows (bf16), emitted per-qb as soon as that
      qb's 4 heads are normalized -> interleaves with the next qb's B work so
      the PE never drains between B and C.
"""
import sys

sys.path.insert(0, "/opt/trn_rl_repo")
sys.path.insert(0, "/root/.axon_site")

import numpy as np

N_EMBD = 2048
N_HEAD = 16
HEAD_DIM = 128
B, T = 2, 2048
N_CORES = 8
H_PER_CORE = 4          # heads per core
HD = H_PER_CORE * HEAD_DIM  # 512: per-core q/k/v width
NC_C = N_EMBD // 128    # 16 contraction chunks
NT = T // 128           # 16 token 128-blocks
NQB = T // 512          # 4 q blocks of 512
SCALE = 1.0 / np.sqrt(HEAD_DIM)
LAG = 3                 # deferred-emission lag (chunks in flight)

_CACHE = {}


def _build():
    import concourse.bacc as bacc
    import concourse.mybir as mybir
    import concourse.tile as tile

    f32 = mybir.dt.float32
    f32r = mybir.dt.float32r
    bf16 = mybir.dt.bfloat16
    Exp = mybir.ActivationFunctionType.Exp
    Ident = mybir.ActivationFunctionType.Identity

    nc = bacc.Bacc("TRN2", target_bir_lowering=False, debug=False, num_devices=N_CORES)

    xt_dram = nc.dram_tensor("xt", [N_EMBD, T], bf16, kind="ExternalInput").ap()
    # host-packed: wa[od][p, c*128+j] = W_{q,k}[c*128+p, hh*128+j], od = kind*4+hh
    # (contiguous 4KB rows -> big DMA descriptors; column-slices of W would be
    # 256B descriptors that crawl under fair-share DMA scheduling)
    wa_dram = nc.dram_tensor("wa", [2 * H_PER_CORE, 128, NC_C * 128], bf16,
                             kind="ExternalInput").ap()
    # host-packed: wvp[p, c*512+j] = W_v[c*128+p, j]
    wvp_dram = nc.dram_tensor("wvp", [128, NC_C * HD], bf16,
                              kind="ExternalInput").ap()
    bq_dram = nc.dram_tensor("bq", [HD, 1], f32, kind="ExternalInput").ap()
    bk_dram = nc.dram_tensor("bk", [HD, 1], f32, kind="ExternalInput").ap()
    bv_dram = nc.dram_tensor("bv", [1, HD], f32, kind="ExternalInput").ap()
    wp_dram = nc.dram_tensor("wp", [HD, N_EMBD], bf16, kind="ExternalInput").ap()
    ones_dram = nc.dram_tensor("ones", [128, 1], bf16, kind="ExternalInput").ap()
    mmask_dram = nc.dram_tensor("mmask", [128, 4, 512], bf16,
                                kind="ExternalInput").ap()
    out_dram = nc.dram_tensor("out", [T, N_EMBD], f32, kind="ExternalOutput").ap()

    with tile.TileContext(nc) as tc:
        with tc.tile_pool(name="singles", bufs=1) as singles, \
             tc.tile_pool(name="qk", bufs=1) as qk_pool, \
             tc.tile_pool(name="vh", bufs=1) as vh_pool:

            ones_col = singles.tile([128, 1], bf16)
            nc.sync.dma_start(ones_col[:], ones_dram[:])
            bias_t = singles.tile([128, 2 * H_PER_CORE], f32)
            nc.sync.dma_start(bias_t[:, 0:4], bq_dram.rearrange("(a p) o -> p (a o)", p=128))
            nc.sync.dma_start(bias_t[:, 4:8], bk_dram.rearrange("(a p) o -> p (a o)", p=128))
            bv_t = singles.tile([1, HD], f32)
            nc.sync.dma_start(bv_t[:], bv_dram[:])

            # resident K^T / Q^T per head [128 d, T], f32r
            qt = [qk_pool.tile([128, T], f32r, tag=f"qt{h}", name=f"qt{h}")
                  for h in range(H_PER_CORE)]
            kt = [qk_pool.tile([128, T], f32r, tag=f"kt{h}", name=f"kt{h}")
                  for h in range(H_PER_CORE)]
            # resident V chunks [128 k, (kc, d)] per head, bf16 (PV runs bf16)
            v_h = [vh_pool.tile([128, NT, 128], bf16, tag=f"vh{h}", name=f"vh{h}")
                   for h in range(H_PER_CORE)]

            # ------- Phase A: K/Q projections, then V in [t, d] blocks -------
            with tc.tile_pool(name="xt", bufs=1) as xt_pool, \
                 tc.tile_pool(name="wqkv", bufs=2) as wqkv_pool, \
                 tc.tile_pool(name="wv", bufs=1) as wv_pool, \
                 tc.tile_pool(name="psA", bufs=2, space="PSUM") as psA:
                # k first (B needs full K per head), then q
                od_order = [(1, hh) for hh in range(H_PER_CORE)] + \
                           [(0, hh) for hh in range(H_PER_CORE)]

                w_tiles = {}

                def issue_w(oi, eng):
                    # host pre-packed contiguous rows, split into 4 partition
                    # chunks: one DMA engine serves ~22.5 GB/s, so a single
                    # 512KB transfer takes ~23us — 4 concurrent chunks cut
                    # that to ~6us
                    w = wqkv_pool.tile([128, NC_C, 128], bf16, tag="wqkv",
                                       name=f"w{oi}")
                    kind, hh = od_order[oi]
                    od = kind * H_PER_CORE + hh
                    src = wa_dram[od].rearrange("p (c j) -> p c j", j=128)
                    for s in range(4):
                        eng.dma_start(w[s * 32:(s + 1) * 32], src[s * 32:(s + 1) * 32])
                    w_tiles[oi] = w

                # w0/w1 before the bulk x transfer; later ods JIT via the
                # scalar queue (gated behind evac work -> don't steal early
                # DMA bandwidth from the x stream)
                issue_w(0, nc.sync)
                issue_w(1, nc.sync)
                # x tiles on the gpsimd DMA queue: early tiles split across
                # engines for latency; bulk tiles issued after the library
                # preload so the first tiles aren't bandwidth-starved by
                # per-packet fair share
                xt = [xt_pool.tile([128, T], bf16, tag=f"xt{c}", name=f"xt{c}")
                      for c in range(NC_C)]

                def xdma(c, nsplit):
                    src = xt_dram[c * 128:(c + 1) * 128, :]
                    step = 128 // nsplit
                    for s in range(nsplit):
                        nc.gpsimd.dma_start(xt[c][s * step:(s + 1) * step, :],
                                            src[s * step:(s + 1) * step, :])

                for c in range(2):
                    xdma(c, 4)
                for c in range(2, 6):
                    xdma(c, 2)
                for c in range(6, 8):
                    xdma(c, 1)
                # broadcast the V bias to all partitions: needed by the V-phase
                # evac, and doubles as the GPSIMD library preload (first custom
                # ISA op pays a multi-us Q7 program fetch). Its ~7us also
                # delays the bulk-x configs below, prioritizing early tiles.
                bvb = singles.tile([128, HD], f32)
                nc.gpsimd.partition_broadcast(bvb[:], bv_t[:])
                for c in range(8, NC_C):
                    xdma(c, 1)

                def evac_od(oi, psums):
                    kind, hh = od_order[oi]
                    od = kind * H_PER_CORE + hh
                    dst = (qt[hh], kt[hh])[kind]
                    for tqb in range(NQB):
                        nc.scalar.activation(
                            dst[:, tqb * 512:(tqb + 1) * 512],
                            psums[tqb][:], Ident,
                            bias=bias_t[:, od:od + 1], scale=1.0)

                wv_all = wv_pool.tile([128, NC_C, HD], bf16, tag="wv",
                                      name="wv_all")
                # first two ods c-interleaved (their 8 psums fill all banks):
                # ~1.7us of PE work per x-tile arrival keeps the PE fed while
                # the x stream is the limiter
                ps_pair = []
                for oi in (0, 1):
                    ps_pair.append([psA.tile([128, 512], f32, tag=f"qk{t}",
                                             name=f"pp{oi}_{t}")
                                    for t in range(NQB)])
                for c in range(NC_C):
                    for oi in (0, 1):
                        w = w_tiles[oi]
                        for tqb in range(NQB):
                            nc.tensor.matmul(
                                ps_pair[oi][tqb][:], w[:, c, :],
                                xt[c][:, tqb * 512:(tqb + 1) * 512],
                                start=(c == 0), stop=(c == NC_C - 1),
                            )
                for oi in (0, 1):
                    evac_od(oi, ps_pair[oi])
                    del w_tiles[oi]

                issue_w(2, nc.sync)
                issue_w(3, nc.scalar)
                for oi in range(2, len(od_order)):
                    kind, hh = od_order[oi]
                    if oi + 2 < len(od_order):
                        issue_w(oi + 2, nc.scalar)
                    if oi == 3:
                        # Wv row-blocks: emitted here so the scalar SEQ gates
                        # the 2MB transfer behind mid-A evac work; 4-way split
                        # so it finishes in ~6us once started
                        wv_src = wvp_dram.rearrange("p (c j) -> p c j", j=HD)
                        for s in range(4):
                            nc.scalar.dma_start(
                                wv_all[s * 32:(s + 1) * 32],
                                wv_src[s * 32:(s + 1) * 32])
                    od = kind * H_PER_CORE + hh
                    psums = []
                    for tqb in range(NQB):
                        p = psA.tile([128, 512], f32, tag=f"qk{tqb}",
                                     name=f"qk{od}_{tqb}")
                        psums.append(p)
                    for c in range(NC_C):
                        w = w_tiles[oi]
                        for tqb in range(NQB):
                            nc.tensor.matmul(
                                psums[tqb][:], w[:, c, :],
                                xt[c][:, tqb * 512:(tqb + 1) * 512],
                                start=(c == 0), stop=(c == NC_C - 1),
                            )
                    del w_tiles[oi]
                    evac_od(oi, psums)

                # V phase: out[t, d] blocks — stationary xt slice, moving Wv
                # row-block. Evacuates straight into the [k, d] layout the PV
                # matmul wants (no transpose), bias added during the evac.
                for tb in range(NT):
                    pv = psA.tile([128, 512], f32, tag=f"qk{tb % 4}",
                                  name=f"pv{tb}")
                    for c in range(NC_C):
                        nc.tensor.matmul(
                            pv[:], xt[c][:, tb * 128:(tb + 1) * 128],
                            wv_all[:, c, :],
                            start=(c == 0), stop=(c == NC_C - 1),
                        )
                    for h in range(H_PER_CORE):
                        with nc.allow_low_precision(reason="v evac f32r"):
                            nc.vector.tensor_add(
                                v_h[h][:, tb, :],
                                pv[:, h * 128:(h + 1) * 128],
                                bvb[:, h * 128:(h + 1) * 128])

            # ---------------- Phases B & C (interleaved) ----------------
            with tc.tile_pool(name="wp", bufs=1) as wp_pool, \
                 tc.tile_pool(name="ytcp", bufs=1) as ytc_pool, \
                 tc.tile_pool(name="bconst", bufs=1) as bconst, \
                 tc.tile_pool(name="pt", bufs=6) as pt_pool, \
                 tc.tile_pool(name="ptm", bufs=4) as ptm_pool, \
                 tc.tile_pool(name="small", bufs=3) as small_pool, \
                 tc.tile_pool(name="oev", bufs=4) as oev_pool, \
                 tc.tile_pool(name="psM", bufs=5, space="PSUM") as psM, \
                 tc.tile_pool(name="psY", bufs=2, space="PSUM") as psY, \
                 tc.tile_pool(name="psS", bufs=1, space="PSUM") as psS:

                mmask = bconst.tile([128, 4, 512], bf16)
                nc.sync.dma_start(mmask[:], mmask_dram[:])
                wp_t = []
                for h in range(H_PER_CORE):
                    w = wp_pool.tile([128, N_EMBD], bf16, tag=f"wp{h}", name=f"wp{h}")
                    nc.sync.dma_start(w[:], wp_dram[h * 128:(h + 1) * 128, :])
                    wp_t.append(w)
                ytc = [ytc_pool.tile([128, 512], bf16, tag=f"ytc{i}", name=f"ytc{i}")
                       for i in range(H_PER_CORE * NQB)]

                deferred = []

                def flush(keep):
                    while len(deferred) > keep:
                        deferred.pop(0)()

                # qb1 first (its short-tail blocks hide behind qb2/qb3 work),
                # qb0 last so the final C tail is the small 4-block one and
                # qb0's B hides C(qb3)'s flushes
                for qi, qb in enumerate((1, 2, 3, 0)):
                    for h in range(H_PER_CORE):
                        i = h * NQB + qb
                        nkc = 4 * (qb + 1)
                        # diagonal (masked) chunks first: their exp->mask chain
                        # is longer; clean chunks behind them keep the PE fed
                        kc_order = list(range(4 * qb, 4 * qb + 4)) + list(range(4 * qb))
                        yt_ps = psY.tile([128, 512], f32, tag="yt", name=f"yt{h}_{qb}")
                        sum_ps = psS.tile([1, 512], f32, tag="sum", name=f"sum{h}_{qb}")
                        for idx, kc in enumerate(kc_order):
                            st = psM.tile([128, 512], f32, tag="mm",
                                          name=f"st{h}_{qb}_{kc}")
                            nc.tensor.matmul(
                                st[:], kt[h][:, kc * 128:(kc + 1) * 128],
                                qt[h][:, qb * 512:(qb + 1) * 512],
                                start=True, stop=True)
                            pt = pt_pool.tile([128, 512], bf16, tag="pt",
                                              name=f"pt{h}_{qb}_{kc}")
                            with nc.allow_low_precision(reason="exp bf16"):
                                nc.scalar.activation(pt[:], st[:], Exp, scale=SCALE)
                            if kc >= 4 * qb:  # diagonal: multiplicative causal mask
                                o = kc - 4 * qb
                                ptm = ptm_pool.tile([128, 512], bf16, tag="ptm",
                                                    name=f"ptm{h}_{qb}_{kc}")
                                with nc.allow_low_precision(reason="mask mul bf16"):
                                    nc.vector.tensor_mul(ptm[:], pt[:], mmask[:, o, :])
                                src = ptm
                            else:
                                src = pt

                            def consume(src=src, yt_ps=yt_ps, sum_ps=sum_ps, kc=kc,
                                        idx=idx, nkc=nkc, h_=h,
                                        last=(idx == nkc - 1)):
                                nc.tensor.matmul(
                                    yt_ps[:], v_h[h_][:, kc, :], src[:],
                                    start=(idx == 0), stop=last)
                                nc.tensor.matmul(
                                    sum_ps[:], ones_col[:], src[:],
                                    start=(idx == 0), stop=last)

                            deferred.append(consume)
                            flush(keep=LAG)

                        def tail(i=i, yt_ps=yt_ps, sum_ps=sum_ps, h_=h, qb_=qb):
                            rinv = small_pool.tile([1, 512], f32, tag="rinv",
                                                   name=f"ri{h_}_{qb_}")
                            nc.vector.reciprocal_approx_fast(rinv[:], sum_ps[:])
                            rbc = small_pool.tile([128, 512], f32, tag="rbc",
                                                  name=f"rb{h_}_{qb_}")
                            nc.gpsimd.partition_broadcast(rbc[:], rinv[:])
                            with nc.allow_low_precision(reason="softmax norm bf16"):
                                nc.vector.tensor_mul(ytc[i][:], yt_ps[:], rbc[:])

                        deferred.append(tail)
                        flush(keep=LAG)

                    # C for this qb: emitted now, drains interleaved with the
                    # next qb's B chunks via the lag queue
                    for tb in range(qb * 4, qb * 4 + 4):
                        ts = (tb % 4) * 128
                        oev = oev_pool.tile([128, N_EMBD], f32, tag="oev",
                                            name=f"oev{tb}")
                        for ob in range(4):
                            def cblock(tb=tb, ts=ts, ob=ob, qb_=qb, oev=oev):
                                po = psM.tile([128, 512], f32, tag="mm",
                                              name=f"po{tb}_{ob}")
                                for h in range(H_PER_CORE):
                                    nc.tensor.matmul(
                                        po[:],
                                        ytc[h * NQB + qb_][:, ts:ts + 128],
                                        wp_t[h][:, ob * 512:(ob + 1) * 512],
                                        start=(h == 0), stop=(h == H_PER_CORE - 1))
                                if ob % 2 == 0:
                                    nc.scalar.copy(
                                        oev[:, ob * 512:(ob + 1) * 512], po[:])
                                else:
                                    nc.vector.tensor_copy(
                                        oev[:, ob * 512:(ob + 1) * 512], po[:])
                                if ob == 3:
                                    nc.sync.dma_start(
                                        out_dram[tb * 128:(tb + 1) * 128, :], oev[:])

                            deferred.append(cblock)
                            if qi < NQB - 1:
                                flush(keep=LAG)
                flush(keep=0)

    nc.compile()
    return nc


def _consts():
    # mask[k_local, o, q_local] = 1 iff q_local >= o*128 + k_local
    # (chunk kc = 4*qb + o; keep iff global q >= global k, qb-independent)
    kk = np.arange(128)[:, None, None]
    oo = np.arange(4)[None, :, None]
    qq = np.arange(512)[None, None, :]
    mmask = (qq >= oo * 128 + kk).astype(np.float32)
    import ml_dtypes
    bf16 = ml_dtypes.bfloat16
    return {
        "ones": np.ones((128, 1), np.float32).astype(bf16),
        "mmask": mmask.astype(bf16),
    }


def _run(inputs, trace=False):
    import ml_dtypes
    from concourse.bass_utils import run_bass_kernel_spmd

    bf16 = ml_dtypes.bfloat16
    if "nc" not in _CACHE:
        _CACHE["nc"] = _build()
    nc = _CACHE["nc"]

    x = np.asarray(inputs["x"], dtype=np.float32)
    W_attn = np.asarray(inputs["W_attn"], dtype=np.float32)
    b_attn = np.asarray(inputs["b_attn"], dtype=np.float32)
    W_proj = np.asarray(inputs["W_proj"], dtype=np.float32)
    b_proj = np.asarray(inputs["b_proj"], dtype=np.float32)

    consts = _consts()
    xt_b = [np.ascontiguousarray(x[b].T).astype(bf16) for b in range(B)]
    in_maps = []
    for m in range(N_CORES):
        b, g = m // 4, m % 4
        cs = g * HD
        # wa[od] = [128 p, 16 c * 128 j] with od = kind*4+hh, kind 0=q / 1=k
        wa = np.empty((2 * H_PER_CORE, 128, NC_C * 128), dtype=bf16)
        for kind, base in ((0, cs), (1, N_EMBD + cs)):
            Ws = W_attn[:, base:base + HD]  # [2048, 512] f32
            for hh in range(H_PER_CORE):
                blk = Ws[:, hh * 128:(hh + 1) * 128].reshape(NC_C, 128, 128)
                wa[kind * H_PER_CORE + hh] = blk.transpose(1, 0, 2).reshape(
                    128, NC_C * 128).astype(bf16)
        Wv = W_attn[:, 2 * N_EMBD + cs:2 * N_EMBD + cs + HD]
        wvp = Wv.reshape(NC_C, 128, HD).transpose(1, 0, 2).reshape(
            128, NC_C * HD).astype(bf16)
        im = {
            "xt": xt_b[b],
            "wa": wa,
            "wvp": np.ascontiguousarray(wvp),
            "bq": np.ascontiguousarray(b_attn[cs:cs + HD].reshape(HD, 1)),
            "bk": np.ascontiguousarray(
                b_attn[N_EMBD + cs:N_EMBD + cs + HD].reshape(HD, 1)),
            "bv": np.ascontiguousarray(
                b_attn[2 * N_EMBD + cs:2 * N_EMBD + cs + HD].reshape(1, HD)),
            "wp": np.ascontiguousarray(W_proj[cs:cs + HD, :]).astype(bf16),
        }
        im.update(consts)
        in_maps.append(im)

    res = run_bass_kernel_spmd(nc, in_maps, list(range(N_CORES)), trace=trace)
    out = np.zeros((B, T, N_EMBD), dtype=np.float32)
    for m in range(N_CORES):
        out[m // 4] += res.results[m]["out"]
    out += b_proj
    return out, res


def kernel(**inputs) -> np.ndarray:
    out, _ = _run(inputs, trace=False)
    return out


# revision 35
# speedup vs baseline: 1.0283x; 1.0168x over previous
"""Causal self-attention (B=2, T=2048, C=2048, H=16, D=128) on 8 TRN2 NeuronCores.

Sharding: 8 cores = 2 batches x 4 head-groups (4 heads each).
Core m: batch b = m // 4, heads [4g, 4g+4) with g = m % 4.
  - c_attn columns split by head (tensor parallel), c_proj rows split by head.
  - Each core returns a partial projection output; host sums the 4 partials
    per batch and adds b_proj (the unshard step for row-parallel c_proj).

Host prep: x is transposed on the host (layout prep) and cast to bf16, so the
device skips the on-chip transpose entirely. W_attn/W_proj also bf16.

Per-core pipeline:
  A:  QT/KT/VT = (x @ W)^T via bf16 matmuls (stationary W chunk, moving xT),
      evacuated by ACT straight into SBUF-resident f32r tiles (no DRAM scratch).
      V goes to bf16 staging tiles for the transpose step.
  T:  per head: transpose VT -> V chunks [k, d] via wide-identity bf16 matmuls.
  B:  qb-outer, head-inner flash-ish attention: ST = K Q^T chunk [128k, 512q]
      -> exp on ACT -> (mask-mul on DVE for the 4 diagonal chunks, processed
      FIRST so their longer chain hides behind clean chunks) -> yT += V^T P,
      sums += ones^T P on PE. Tail: 1/sum via DVE reciprocal (no Ln -> no ACT
      table thrash), broadcast via GpSimd partition_broadcast, normalize on DVE
      into bf16 ytc tiles. Emission software-pipelined with a lag queue.
  C:  out = concat_heads(y) @ Wp_rows (bf16), emitted per-qb as soon as that
      qb's 4 heads are normalized -> interleaves with the next qb's B work so
      the PE never drains between B and C.
"""
import sys

sys.path.insert(0, "/opt/trn_rl_repo")
sys.path.insert(0, "/root/.axon_site")

import numpy as np

N_EMBD = 2048
N_HEAD = 16
HEAD_DIM = 128
B, T = 2, 2048
N_CORES = 8
H_PER_CORE = 4          # heads per core
HD = H_PER_CORE * HEAD_DIM  # 512: per-core q/k/v width
NC_C = N_EMBD // 128    # 16 contraction chunks
NT = T // 128           # 16 token 128-blocks
NQB = T // 512          # 4 q blocks of 512
SCALE = 1.0 / np.sqrt(HEAD_DIM)
LAG = 3                 # deferred-emission lag (chunks in flight)

_CACHE = {}


def _build():
    import concourse.bacc as bacc
    import concourse.mybir as mybir
    import concourse.tile as tile

    f32 = mybir.dt.float32
    f32r = mybir.dt.float32r
    bf16 = mybir.dt.bfloat16
    Exp = mybir.ActivationFunctionType.Exp
    Ident = mybir.ActivationFunctionType.Identity

    nc = bacc.Bacc("TRN2", target_bir_lowering=False, debug=False, num_devices=N_CORES)

    xt_dram = nc.dram_tensor("xt", [N_EMBD, T], bf16, kind="ExternalInput").ap()
    # host-packed: wa[od][p, c*128+j] = W_{q,k}[c*128+p, hh*128+j], od = kind*4+hh
    # (contiguous 4KB rows -> big DMA descriptors; column-slices of W would be
    # 256B descriptors that crawl under fair-share DMA scheduling)
    wa_dram = nc.dram_tensor("wa", [2 * H_PER_CORE, 128, NC_C * 128], bf16,
                             kind="ExternalInput").ap()
    # host-packed: wvp[p, c*512+j] = W_v[c*128+p, j]
    wvp_dram = nc.dram_tensor("wvp", [128, NC_C * HD], bf16,
                              kind="ExternalInput").ap()
    bq_dram = nc.dram_tensor("bq", [HD, 1], f32, kind="ExternalInput").ap()
    bk_dram = nc.dram_tensor("bk", [HD, 1], f32, kind="ExternalInput").ap()
    bv_dram = nc.dram_tensor("bv", [1, HD], f32, kind="ExternalInput").ap()
    wp_dram = nc.dram_tensor("wp", [HD, N_EMBD], bf16, kind="ExternalInput").ap()
    ones_dram = nc.dram_tensor("ones", [128, 1], bf16, kind="ExternalInput").ap()
    mmask_dram = nc.dram_tensor("mmask", [128, 4, 512], bf16,
                                kind="ExternalInput").ap()
    out_dram = nc.dram_tensor("out", [T, N_EMBD], f32, kind="ExternalOutput").ap()

    with tile.TileContext(nc) as tc:
        with tc.tile_pool(name="singles", bufs=1) as singles, \
             tc.tile_pool(name="qk", bufs=1) as qk_pool, \
             tc.tile_pool(name="vh", bufs=1) as vh_pool:

            ones_col = singles.tile([128, 1], bf16)
            nc.sync.dma_start(ones_col[:], ones_dram[:])
            bias_t = singles.tile([128, 2 * H_PER_CORE], f32)
            nc.sync.dma_start(bias_t[:, 0:4], bq_dram.rearrange("(a p) o -> p (a o)", p=128))
            nc.sync.dma_start(bias_t[:, 4:8], bk_dram.rearrange("(a p) o -> p (a o)", p=128))
            bv_t = singles.tile([1, HD], f32)
            nc.sync.dma_start(bv_t[:], bv_dram[:])

            # resident K^T / Q^T per head [128 d, T], f32r
            qt = [qk_pool.tile([128, T], f32r, tag=f"qt{h}", name=f"qt{h}")
                  for h in range(H_PER_CORE)]
            kt = [qk_pool.tile([128, T], f32r, tag=f"kt{h}", name=f"kt{h}")
                  for h in range(H_PER_CORE)]
            # resident V chunks [128 k, (kc, d)] per head, bf16 (PV runs bf16)
            v_h = [vh_pool.tile([128, NT, 128], bf16, tag=f"vh{h}", name=f"vh{h}")
                   for h in range(H_PER_CORE)]

            # ------- Phase A: K/Q projections, then V in [t, d] blocks -------
            with tc.tile_pool(name="xt", bufs=1) as xt_pool, \
                 tc.tile_pool(name="wqkv", bufs=3) as wqkv_pool, \
                 tc.tile_pool(name="wv", bufs=1) as wv_pool, \
                 tc.tile_pool(name="psA", bufs=2, space="PSUM") as psA:
                # k first (B needs full K per head), then q
                od_order = [(1, hh) for hh in range(H_PER_CORE)] + \
                           [(0, hh) for hh in range(H_PER_CORE)]

                w_tiles = {}

                def issue_w(oi, eng):
                    # host pre-packed contiguous rows, split into 4 partition
                    # chunks: one DMA engine serves ~22.5 GB/s, so a single
                    # 512KB transfer takes ~23us — 4 concurrent chunks cut
                    # that to ~6us
                    w = wqkv_pool.tile([128, NC_C, 128], bf16, tag="wqkv",
                                       name=f"w{oi}")
                    kind, hh = od_order[oi]
                    od = kind * H_PER_CORE + hh
                    src = wa_dram[od].rearrange("p (c j) -> p c j", j=128)
                    for s in range(4):
                        eng.dma_start(w[s * 32:(s + 1) * 32], src[s * 32:(s + 1) * 32])
                    w_tiles[oi] = w

                # w0/w1 before the bulk x transfer; later ods JIT via the
                # scalar queue (gated behind evac work -> don't steal early
                # DMA bandwidth from the x stream)
                issue_w(0, nc.sync)
                issue_w(1, nc.sync)
                # x tiles on the gpsimd DMA queue: early tiles split across
                # engines for latency; bulk tiles issued after the library
                # preload so the first tiles aren't bandwidth-starved by
                # per-packet fair share
                xt = [xt_pool.tile([128, T], bf16, tag=f"xt{c}", name=f"xt{c}")
                      for c in range(NC_C)]

                def xdma(c, nsplit):
                    src = xt_dram[c * 128:(c + 1) * 128, :]
                    step = 128 // nsplit
                    for s in range(nsplit):
                        nc.gpsimd.dma_start(xt[c][s * step:(s + 1) * step, :],
                                            src[s * step:(s + 1) * step, :])

                for c in range(4):
                    xdma(c, 4)
                for c in range(4, 8):
                    xdma(c, 2)
                for c in range(8, NC_C):
                    xdma(c, 1)
                # broadcast the V bias to all partitions: needed by the V-phase
                # evac, and doubles as the GPSIMD library preload (first custom
                # ISA op pays a multi-us Q7 program fetch). Placed after the
                # x configs so the load doesn't delay the x stream.
                bvb = singles.tile([128, HD], f32)
                nc.gpsimd.partition_broadcast(bvb[:], bv_t[:])

                def evac_od(oi, psums):
                    kind, hh = od_order[oi]
                    od = kind * H_PER_CORE + hh
                    dst = (qt[hh], kt[hh])[kind]
                    for tqb in range(NQB):
                        nc.scalar.activation(
                            dst[:, tqb * 512:(tqb + 1) * 512],
                            psums[tqb][:], Ident,
                            bias=bias_t[:, od:od + 1], scale=1.0)

                wv_all = wv_pool.tile([128, NC_C, HD], bf16, tag="wv",
                                      name="wv_all")
                # first two ods c-interleaved (their 8 psums fill all banks):
                # ~1.7us of PE work per x-tile arrival keeps the PE fed while
                # the x stream is the limiter
                ps_pair = []
                for oi in (0, 1):
                    ps_pair.append([psA.tile([128, 512], f32, tag=f"qk{t}",
                                             name=f"pp{oi}_{t}")
                                    for t in range(NQB)])
                for c in range(NC_C):
                    for oi in (0, 1):
                        w = w_tiles[oi]
                        for tqb in range(NQB):
                            nc.tensor.matmul(
                                ps_pair[oi][tqb][:], w[:, c, :],
                                xt[c][:, tqb * 512:(tqb + 1) * 512],
                                start=(c == 0), stop=(c == NC_C - 1),
                            )
                for oi in (0, 1):
                    evac_od(oi, ps_pair[oi])
                    del w_tiles[oi]

                issue_w(2, nc.sync)
                issue_w(3, nc.scalar)
                for oi in range(2, len(od_order)):
                    kind, hh = od_order[oi]
                    if oi + 2 < len(od_order):
                        issue_w(oi + 2, nc.scalar)
                    if oi == 3:
                        # Wv row-blocks: emitted here so the scalar SEQ gates
                        # the 2MB transfer behind mid-A evac work; 4-way split
                        # so it finishes in ~6us once started
                        wv_src = wvp_dram.rearrange("p (c j) -> p c j", j=HD)
                        for s in range(4):
                            nc.scalar.dma_start(
                                wv_all[s * 32:(s + 1) * 32],
                                wv_src[s * 32:(s + 1) * 32])
                    od = kind * H_PER_CORE + hh
                    psums = []
                    for tqb in range(NQB):
                        p = psA.tile([128, 512], f32, tag=f"qk{tqb}",
                                     name=f"qk{od}_{tqb}")
                        psums.append(p)
                    for c in range(NC_C):
                        w = w_tiles[oi]
                        for tqb in range(NQB):
                            nc.tensor.matmul(
                                psums[tqb][:], w[:, c, :],
                                xt[c][:, tqb * 512:(tqb + 1) * 512],
                                start=(c == 0), stop=(c == NC_C - 1),
                            )
                    del w_tiles[oi]
                    evac_od(oi, psums)

                # V phase: out[t, d] blocks — stationary xt slice, moving Wv
                # row-block. Evacuates straight into the [k, d] layout the PV
                # matmul wants (no transpose), bias added during the evac.
                for tb in range(NT):
                    pv = psA.tile([128, 512], f32, tag=f"qk{tb % 4}",
                                  name=f"pv{tb}")
                    for c in range(NC_C):
                        nc.tensor.matmul(
                            pv[:], xt[c][:, tb * 128:(tb + 1) * 128],
                            wv_all[:, c, :],
                            start=(c == 0), stop=(c == NC_C - 1),
                        )
                    for h in range(H_PER_CORE):
                        with nc.allow_low_precision(reason="v evac f32r"):
                            nc.vector.tensor_add(
                                v_h[h][:, tb, :],
                                pv[:, h * 128:(h + 1) * 128],
                                bvb[:, h * 128:(h + 1) * 128])

            # ---------------- Phases B & C (interleaved) ----------------
            with tc.tile_pool(name="wp", bufs=1) as wp_pool, \
                 tc.tile_pool(name="ytcp", bufs=1) as ytc_pool, \
                 tc.tile_pool(name="bconst", bufs=1) as bconst, \
                 tc.tile_pool(name="pt", bufs=6) as pt_pool, \
                 tc.tile_pool(name="ptm", bufs=4) as ptm_pool, \
                 tc.tile_pool(name="small", bufs=3) as small_pool, \
                 tc.tile_pool(name="oev", bufs=4) as oev_pool, \
                 tc.tile_pool(name="psM", bufs=5, space="PSUM") as psM, \
                 tc.tile_pool(name="psY", bufs=2, space="PSUM") as psY, \
                 tc.tile_pool(name="psS", bufs=1, space="PSUM") as psS:

                mmask = bconst.tile([128, 4, 512], bf16)
                nc.sync.dma_start(mmask[:], mmask_dram[:])
                wp_t = []
                for h in range(H_PER_CORE):
                    w = wp_pool.tile([128, N_EMBD], bf16, tag=f"wp{h}", name=f"wp{h}")
                    nc.sync.dma_start(w[:], wp_dram[h * 128:(h + 1) * 128, :])
                    wp_t.append(w)
                ytc = [ytc_pool.tile([128, 512], bf16, tag=f"ytc{i}", name=f"ytc{i}")
                       for i in range(H_PER_CORE * NQB)]

                deferred = []

                def flush(keep):
                    while len(deferred) > keep:
                        deferred.pop(0)()

                # qb1 first (its short-tail blocks hide behind qb2/qb3 work),
                # qb0 last so the final C tail is the small 4-block one and
                # qb0's B hides C(qb3)'s flushes
                for qi, qb in enumerate((1, 2, 3, 0)):
                    for h in range(H_PER_CORE):
                        i = h * NQB + qb
                        nkc = 4 * (qb + 1)
                        # diagonal (masked) chunks first: their exp->mask chain
                        # is longer; clean chunks behind them keep the PE fed
                        kc_order = list(range(4 * qb, 4 * qb + 4)) + list(range(4 * qb))
                        yt_ps = psY.tile([128, 512], f32, tag="yt", name=f"yt{h}_{qb}")
                        sum_ps = psS.tile([1, 512], f32, tag="sum", name=f"sum{h}_{qb}")
                        for idx, kc in enumerate(kc_order):
                            st = psM.tile([128, 512], f32, tag="mm",
                                          name=f"st{h}_{qb}_{kc}")
                            nc.tensor.matmul(
                                st[:], kt[h][:, kc * 128:(kc + 1) * 128],
                                qt[h][:, qb * 512:(qb + 1) * 512],
                                start=True, stop=True)
                            pt = pt_pool.tile([128, 512], bf16, tag="pt",
                                              name=f"pt{h}_{qb}_{kc}")
                            with nc.allow_low_precision(reason="exp bf16"):
                                nc.scalar.activation(pt[:], st[:], Exp, scale=SCALE)
                            if kc >= 4 * qb:  # diagonal: multiplicative causal mask
                                o = kc - 4 * qb
                                ptm = ptm_pool.tile([128, 512], bf16, tag="ptm",
                                                    name=f"ptm{h}_{qb}_{kc}")
                                with nc.allow_low_precision(reason="mask mul bf16"):
                                    nc.vector.tensor_mul(ptm[:], pt[:], mmask[:, o, :])
                                src = ptm
                            else:
                                src = pt

                            def consume(src=src, yt_ps=yt_ps, sum_ps=sum_ps, kc=kc,
                                        idx=idx, nkc=nkc, h_=h,
                                        last=(idx == nkc - 1)):
                                nc.tensor.matmul(
                                    yt_ps[:], v_h[h_][:, kc, :], src[:],
                                    start=(idx == 0), stop=last)
                                nc.tensor.matmul(
                                    sum_ps[:], ones_col[:], src[:],
                                    start=(idx == 0), stop=last)

                            deferred.append(consume)
                            flush(keep=LAG)

                        def tail(i=i, yt_ps=yt_ps, sum_ps=sum_ps, h_=h, qb_=qb):
                            rinv = small_pool.tile([1, 512], f32, tag="rinv",
                                                   name=f"ri{h_}_{qb_}")
                            nc.vector.reciprocal_approx_fast(rinv[:], sum_ps[:])
                            rbc = small_pool.tile([128, 512], f32, tag="rbc",
                                                  name=f"rb{h_}_{qb_}")
                            nc.gpsimd.partition_broadcast(rbc[:], rinv[:])
                            with nc.allow_low_precision(reason="softmax norm bf16"):
                                nc.vector.tensor_mul(ytc[i][:], yt_ps[:], rbc[:])

                        deferred.append(tail)
                        flush(keep=LAG)

                    # C for this qb: emitted now, drains interleaved with the
                    # next qb's B chunks via the lag queue
                    for tb in range(qb * 4, qb * 4 + 4):
                        ts = (tb % 4) * 128
                        oev = oev_pool.tile([128, N_EMBD], f32, tag="oev",
                                            name=f"oev{tb}")
                        for ob in range(4):
                            def cblock(tb=tb, ts=ts, ob=ob, qb_=qb, oev=oev):
                                po = psM.tile([128, 512], f32, tag="mm",
                                              name=f"po{tb}_{ob}")
                                for h in range(H_PER_CORE):
                                    nc.tensor.matmul(
                                        po[:],
                                        ytc[h * NQB + qb_][:, ts:ts + 128],
                                        wp_t[h][:, ob * 512:(ob + 1) * 512],
                                        start=(h == 0), stop=(h == H_PER_CORE - 1))
                                if ob % 2 == 0:
                                    nc.scalar.copy(
                                        oev[:, ob * 512:(ob + 1) * 512], po[:])
                                else:
                                    nc.vector.tensor_copy(
                                        oev[:, ob * 512:(ob + 1) * 512], po[:])
                                if ob == 3:
                                    nc.sync.dma_start(
                                        out_dram[tb * 128:(tb + 1) * 128, :], oev[:])

                            deferred.append(cblock)
                            if qi < NQB - 1:
                                flush(keep=LAG)
                flush(keep=0)

    nc.compile()
    return nc


def _consts():
    # mask[k_local, o, q_local] = 1 iff q_local >= o*128 + k_local
    # (chunk kc = 4*qb + o; keep iff global q >= global k, qb-independent)
    kk = np.arange(128)[:, None, None]
    oo = np.arange(4)[None, :, None]
    qq = np.arange(512)[None, None, :]
    mmask = (qq >= oo * 128 + kk).astype(np.float32)
    import ml_dtypes
    bf16 = ml_dtypes.bfloat16
    return {
        "ones": np.ones((128, 1), np.float32).astype(bf16),
        "mmask": mmask.astype(bf16),
    }


def _run(inputs, trace=False):
    import ml_dtypes
    from concourse.bass_utils import run_bass_kernel_spmd

    bf16 = ml_dtypes.bfloat16
    if "nc" not in _CACHE:
        _CACHE["nc"] = _build()
    nc = _CACHE["nc"]

    x = np.asarray(inputs["x"], dtype=np.float32)
    W_attn = np.asarray(inputs["W_attn"], dtype=np.float32)
    b_attn = np.asarray(inputs["b_attn"], dtype=np.float32)
    W_proj = np.asarray(inputs["W_proj"], dtype=np.float32)
    b_proj = np.asarray(inputs["b_proj"], dtype=np.float32)

    consts = _consts()
    xt_b = [np.ascontiguousarray(x[b].T).astype(bf16) for b in range(B)]
    in_maps = []
    for m in range(N_CORES):
        b, g = m // 4, m % 4
        cs = g * HD
        # wa[od] = [128 p, 16 c * 128 j] with od = kind*4+hh, kind 0=q / 1=k
        wa = np.empty((2 * H_PER_CORE, 128, NC_C * 128), dtype=bf16)
        for kind, base in ((0, cs), (1, N_EMBD + cs)):
            Ws = W_attn[:, base:base + HD]  # [2048, 512] f32
            for hh in range(H_PER_CORE):
                blk = Ws[:, hh * 128:(hh + 1) * 128].reshape(NC_C, 128, 128)
                wa[kind * H_PER_CORE + hh] = blk.transpose(1, 0, 2).reshape(
                    128, NC_C * 128).astype(bf16)
        Wv = W_attn[:, 2 * N_EMBD + cs:2 * N_EMBD + cs + HD]
        wvp = Wv.reshape(NC_C, 128, HD).transpose(1, 0, 2).reshape(
            128, NC_C * HD).astype(bf16)
        im = {
            "xt": xt_b[b],
            "wa": wa,
            "wvp": np.ascontiguousarray(wvp),
            "bq": np.ascontiguousarray(b_attn[cs:cs + HD].reshape(HD, 1)),
            "bk": np.ascontiguousarray(
                b_attn[N_EMBD + cs:N_EMBD + cs + HD].reshape(HD, 1)),
            "bv": np.ascontiguousarray(
                b_attn[2 * N_EMBD + cs:2 * N_EMBD + cs + HD].reshape(1, HD)),
            "wp": np.ascontiguousarray(W_proj[cs:cs + HD, :]).astype(bf16),
        }
        im.update(consts)
        in_maps.append(im)

    res = run_bass_kernel_spmd(nc, in_maps, list(range(N_CORES)), trace=trace)
    out = np.zeros((B, T, N_EMBD), dtype=np.float32)
    for m in range(N_CORES):
        out[m // 4] += res.results[m]["out"]
    out += b_proj
    return out, res


def kernel(**inputs) -> np.ndarray:
    out, _ = _run(inputs, trace=False)
    return out
